# revision 17
# baseline (speedup 1.0000x reference)
"""Trainium2 Bass kernel for nn_Net_661424963757 (2-layer LIF SNN, 25 steps).

Strategy (8 NeuronCores, data-parallel over batch, shard of 512 rows/core):
  - Host prep (layout only): transpose + bf16 hi/lo split of W1/x, fp16 W2
    (spikes are exact {0,1} in fp16, so mm2 precision = W2's 11 bits), shard x.
  - Device, per core:
      cur1T[h, b] = W1 @ x.T + b1 once, via 3-pass bf16 hi/lo matmuls
      (W1T streamed in 8 chunks), bias folded into the PSUM eviction on ACT.
      25 LIF steps, state = pre-reset membrane m_pre, fused custom DVE op:
        m_pre' = (m_pre - (m_pre > 1)*1) * beta + in
      (reset subtract exact in f32; same rounding pattern as the reference).
      Layer-1 state/compares chunked 4-way along NH for pipelining; spike
      compares: 3 chunks on gpsimd (is_gt -> fp16), 1 chunk + spk2 on ACT
      (sign then relu, exact). Layer-2 matmul (spk1 @ W2.T + b2) in
      transposed [NO, B] layout, single fp16 pass per k-tile accumulated in
      PSUM; bias via a K=2 matmul row (fp16 hi/lo of b2). The layer-2 LIF op
      reads PSUM directly. DMA out spk2 and m_pre2 per step (HWDGE).
  - Host post: mem2 = m_pre2 - spk2 (exact), transpose [NO,B]->[B,NO],
    concat shards.
Cost-model estimate ~348 us/core; PE 268 us, DVE 259 us, Pool 221 us.
"""

import numpy as np
import ml_dtypes

import concourse.bacc as bacc
import concourse.mybir as mybir
import concourse.tile as tile
from concourse.bass_utils import run_bass_kernel_spmd
from concourse.dve_spec import Spec, Src0, Src1, C0, C1, C2, lower
from concourse.dve_uop import DveOpSpec
import concourse.dve_ops as dve_ops

BETA = 0.95
T = 25
B, NI, NH, NO = 4096, 1024, 2048, 256
NCORES = 8
BL = B // NCORES                      # 512 batch rows per core
KI, KH, MH, MO = NI // 128, NH // 128, NH // 128, NO // 128   # 8, 16, 16, 2
BF16 = mybir.dt.bfloat16
F16 = mybir.dt.float16
F32 = mybir.dt.float32


def _lif_ref(in0, in1, s0, s1, imm2):
    # (in0 - (in0 > s0)*s0) * s1 + in1
    in0 = in0.astype(np.float32)
    spk = (in0 > np.float32(s0)).astype(np.float32)
    return (in0 - spk * np.float32(s0)) * np.float32(s1) + in1.astype(np.float32)


def _register_lif_op():
    """Fused LIF step, state kept as pre-reset membrane m_pre:
       m_pre' = (m_pre - (m_pre > thresh)*thresh) * beta + in1
    The reset subtract is exact in f32 (thresh=1), then one beta-multiply
    rounding and one add rounding - the same rounding pattern as the
    reference's  beta*(m_pre - spk) + cur.
    """
    name = "LIF_STEPB_ANT"
    if name in dve_ops._SUB_OPCODE_FOR_NAME:
        return next(op for op in dve_ops.OPS if op.name == name)
    g = Src0 > C0
    spec = Spec(body=(Src0 - g * C0) * C1 + Src1, reference=_lif_ref)
    opcode = dve_ops._CUSTOM_DVE_ROW_BASE + len(dve_ops.OPS)
    shas = {}
    for ver in ("v3", "v4"):
        s = DveOpSpec(name=name, opcode=opcode, uops=lower(spec, ver=ver), rd1_en=True)
        shas[ver] = s.sha(ver)
    op = dve_ops.DveOp(name, spec, subdim=False, uops_sha=shas)
    dve_ops.OPS.append(op)
    dve_ops.CUSTOM_DVE_SPECS[name] = spec
    dve_ops._SUB_OPCODE_FOR_NAME[name] = opcode
    return op


LIF_OP = _register_lif_op()

_module_cache = {}


def _build_module():
    if "nc" in _module_cache:
        return _module_cache["nc"]

    nc = bacc.Bacc("TRN2", target_bir_lowering=False, debug=False,
                   num_devices=NCORES)

    d_w1h = nc.dram_tensor("w1h", [KI, 128, NH], BF16, kind="ExternalInput")
    d_w1l = nc.dram_tensor("w1l", [KI, 128, NH], BF16, kind="ExternalInput")
    d_xh = nc.dram_tensor("xh", [KI, 128, BL], BF16, kind="ExternalInput")
    d_xl = nc.dram_tensor("xl", [KI, 128, BL], BF16, kind="ExternalInput")
    d_w2h = nc.dram_tensor("w2h", [KH, 128, NO], F16, kind="ExternalInput")
    d_b1 = nc.dram_tensor("b1r", [MH, 128], F32, kind="ExternalInput")
    d_b2 = nc.dram_tensor("b2hl", [2, NO], F16, kind="ExternalInput")
    d_ones = nc.dram_tensor("ones2", [2, BL], F16, kind="ExternalInput")
    d_spk = nc.dram_tensor("spk_rec", [T, NO, BL], F32, kind="ExternalOutput")
    d_m2 = nc.dram_tensor("m2_rec", [T, NO, BL], F32, kind="ExternalOutput")

    ident = mybir.ActivationFunctionType.Identity

    with tile.TileContext(nc) as tc:
        with tc.tile_pool(name="const", bufs=1) as const, \
             tc.tile_pool(name="state", bufs=1) as state, \
             tc.tile_pool(name="outp", bufs=3) as outp:

            t_w2h = const.tile([128, KH, NO], F16)
            t_b2 = const.tile([2, NO], F16)
            t_ones = const.tile([2, BL], F16)
            t_b1 = const.tile([128, MH], F32)
            t_neg1 = const.tile([128, 1], F32)
            nc.vector.memset(t_neg1, -1.0)
            nc.sync.dma_start(out=t_w2h, in_=d_w2h.rearrange("k p o -> p k o"))
            nc.sync.dma_start(out=t_b2, in_=d_b2[:, :])
            nc.sync.dma_start(out=t_ones, in_=d_ones[:, :])
            nc.sync.dma_start(out=t_b1, in_=d_b1.rearrange("m p -> p m"))

            # layer-1 tensors chunked 4-way along NH (4 m-tiles per chunk) so
            # setup, LIF updates, spike compares and matmuls pipeline finely
            NG = 4
            GW = (MH // NG) * BL                    # free width per chunk
            t_cur1 = [state.tile([128, GW], F32, name=f"cur1_{g}")
                      for g in range(NG)]
            t_m1 = [[state.tile([128, GW], F32, name=f"m1_{i}_{g}")
                     for g in range(NG)] for i in (0, 1)]
            t_m2 = [state.tile([128, MO * BL], F32, name=f"m2_{i}") for i in (0, 1)]
            t_spk1 = [[state.tile([128, GW], F16, name=f"spk1_{i}_{g}")
                       for g in range(NG)] for i in (0, 1)]

            for g in range(NG):
                nc.vector.memset(t_m1[0][g], 0.0)
            nc.vector.memset(t_m2[0], 0.0)

            # ---- setup: cur1T = W1 @ x.T + b1, 3-pass bf16 hi/lo ----------
            # W1T streamed in 8 NH-chunks of 256 cols (2 m-tiles each) to fit
            # SBUF; x.T (hi/lo) stays resident.
            with tc.tile_pool(name="setup", bufs=1) as setup, \
                 tc.tile_pool(name="setw", bufs=2) as setw, \
                 tc.tile_pool(name="psum_c", bufs=4, space="PSUM") as psum_c:
                t_xh = setup.tile([128, KI, BL], BF16)
                t_xl = setup.tile([128, KI, BL], BF16)
                nc.sync.dma_start(out=t_xh, in_=d_xh.rearrange("k p b -> p k b"))
                nc.sync.dma_start(out=t_xl, in_=d_xl.rearrange("k p b -> p k b"))

                for g in range(MH // 2):
                    gsl = slice(g * 256, (g + 1) * 256)
                    t_w1h = setw.tile([128, KI, 256], BF16, tag="w1h")
                    t_w1l = setw.tile([128, KI, 256], BF16, tag="w1l")
                    nc.sync.dma_start(
                        out=t_w1h, in_=d_w1h[:, :, gsl].rearrange("k p h -> p k h"))
                    nc.sync.dma_start(
                        out=t_w1l, in_=d_w1l[:, :, gsl].rearrange("k p h -> p k h"))
                    for mm in range(2):
                        m = g * 2 + mm
                        ps = psum_c.tile([128, BL], F32)
                        hs = slice(mm * 128, (mm + 1) * 128)
                        for k in range(KI):
                            nc.tensor.matmul(ps, t_w1h[:, k, hs], t_xh[:, k, :],
                                             start=(k == 0), stop=False)
                            nc.tensor.matmul(ps, t_w1l[:, k, hs], t_xh[:, k, :],
                                             start=False, stop=False)
                            nc.tensor.matmul(ps, t_w1h[:, k, hs], t_xl[:, k, :],
                                             start=False, stop=(k == KI - 1))
                        cg, cm = m // (MH // NG), m % (MH // NG)
                        nc.scalar.activation(t_cur1[cg][:, cm * BL:(cm + 1) * BL],
                                             ps, ident, bias=t_b1[:, m:m + 1],
                                             scale=1.0)

            # ---- 25 LIF steps --------------------------------------------
            KPG = KH // NG                          # k-tiles per chunk
            with tc.tile_pool(name="psum_s", bufs=2, space="PSUM") as psum_s:
                cur1_3d = [t.rearrange("p (s n) -> p s n", s=1) for t in t_cur1]
                for s in range(1, T + 1):
                    p, q = s % 2, (s - 1) % 2
                    for g in range(NG):
                        nc.vector._custom_dve(LIF_OP, out=t_m1[p][g],
                                              in0=t_m1[q][g], in1=cur1_3d[g],
                                              s0=1.0, s1=BETA)
                        if g == NG - 1:
                            # last chunk's spike compare on ACT: sign then
                            # relu, in place in the fp16 tile (exact values)
                            nc.scalar.activation(
                                t_spk1[p][g], t_m1[p][g],
                                mybir.ActivationFunctionType.Sign,
                                bias=t_neg1[:, :])
                            nc.scalar.activation(
                                t_spk1[p][g], t_spk1[p][g],
                                mybir.ActivationFunctionType.Relu)
                        else:
                            nc.gpsimd.tensor_scalar(t_spk1[p][g], t_m1[p][g],
                                                    1.0, None,
                                                    mybir.AluOpType.is_gt)
                    ps = psum_s.tile([128, MO * BL], F32)
                    spk3 = [t.rearrange("p (k n) -> p k n", k=KPG)
                            for t in t_spk1[p]]
                    for mo in range(MO):
                        dst = ps[:, mo * BL:(mo + 1) * BL]
                        osl = slice(mo * 128, (mo + 1) * 128)
                        nc.tensor.matmul(dst, t_b2[:, osl], t_ones,
                                         start=True, stop=False)
                        for k in range(KH):
                            sp = spk3[k // KPG][:, k % KPG, :]
                            nc.tensor.matmul(dst, t_w2h[:, k, osl], sp,
                                             start=False, stop=(k == KH - 1))
                    nc.vector._custom_dve(
                        LIF_OP, out=t_m2[p], in0=t_m2[q],
                        in1=ps.rearrange("p (s n) -> p s n", s=1),
                        s0=1.0, s1=BETA)
                    t_sgn = outp.tile([128, MO * BL], F32, tag="sgn")
                    nc.scalar.activation(t_sgn, t_m2[p],
                                         mybir.ActivationFunctionType.Sign,
                                         bias=t_neg1[:, :])
                    t_spk2 = outp.tile([128, MO * BL], F32, tag="spk2")
                    nc.scalar.activation(t_spk2, t_sgn,
                                         mybir.ActivationFunctionType.Relu)
                    nc.sync.dma_start(
                        out=d_spk[s - 1].rearrange("(mo p) b -> p mo b", p=128),
                        in_=t_spk2.rearrange("p (mo b) -> p mo b", mo=MO))
                    nc.sync.dma_start(
                        out=d_m2[s - 1].rearrange("(mo p) b -> p mo b", p=128),
                        in_=t_m2[p].rearrange("p (mo b) -> p mo b", mo=MO))

    nc.compile()
    _module_cache["nc"] = nc
    return nc


def _split_bf16(a):
    hi = a.astype(ml_dtypes.bfloat16)
    lo = (a - hi.astype(np.float32)).astype(ml_dtypes.bfloat16)
    return hi, lo


def kernel(x, W1, b1, W2, b2, _want_trace=False):
    x = np.asarray(x, np.float32)
    W1 = np.asarray(W1, np.float32)
    b1 = np.asarray(b1, np.float32)
    W2 = np.asarray(W2, np.float32)
    b2 = np.asarray(b2, np.float32)
    nc = _build_module()

    w1h, w1l = _split_bf16(np.ascontiguousarray(W1.T))      # [NI, NH]
    w2h = np.ascontiguousarray(W2.T).astype(np.float16)     # [NH, NO]
    b2h = b2.astype(np.float16)
    b2l = (b2 - b2h.astype(np.float32)).astype(np.float16)
    common = {
        "w1h": np.ascontiguousarray(w1h.reshape(KI, 128, NH)),
        "w1l": np.ascontiguousarray(w1l.reshape(KI, 128, NH)),
        "w2h": np.ascontiguousarray(w2h.reshape(KH, 128, NO)),
        "b1r": np.ascontiguousarray(b1.astype(np.float32).reshape(MH, 128)),
        "b2hl": np.ascontiguousarray(np.stack([b2h, b2l])),
        "ones2": np.ones((2, BL), dtype=np.float16),
    }
    in_maps = []
    for c in range(NCORES):
        xs = np.ascontiguousarray(x[c * BL:(c + 1) * BL].T)  # [NI, BL]
        xh, xl = _split_bf16(xs)
        in_maps.append({
            **common,
            "xh": np.ascontiguousarray(xh.reshape(KI, 128, BL)),
            "xl": np.ascontiguousarray(xl.reshape(KI, 128, BL)),
        })

    out = run_bass_kernel_spmd(nc, in_maps, core_ids=list(range(NCORES)),
                               trace=_want_trace)
    spk = np.stack([r["spk_rec"] for r in out.results])   # [C, T, NO, BL]
    m2r = np.stack([r["m2_rec"] for r in out.results])    # [C, T, NO, BL]

    # device state is the pre-reset membrane; post-reset mem2 = m_pre - spk
    # (exact f32 subtraction since spk is 0/1 and threshold is 1.0)
    mem2 = m2r - spk

    spk_full = spk.transpose(1, 0, 3, 2).reshape(T, B, NO)
    mem2_full = mem2.transpose(1, 0, 3, 2).reshape(T, B, NO)
    if _want_trace:
        kernel._last_result = out
    return spk_full.astype(np.float32), mem2_full.astype(np.float32)
